# revision 9
# baseline (speedup 1.0000x reference)
"""Bass/Tile Trainium2 kernel for nn_Attention3 (additive/Bahdanau attention).

reference:
    q = decoder_hidden @ W_w.T + W_b          # [B, 1, D]
    k = encoder_outputs @ U_w.T + U_b         # [B, L, D]
    scores = tanh(q + k) @ v_w.T + v_b        # [B, L, 1]
    attn = softmax(scores[..., 0])[:, None]   # [B, 1, L]
    context = attn @ encoder_outputs          # [B, 1, D]
    returns (context, attn)

Sharding: data-parallel over batch B=32 across 8 cores (4 batches/core),
weights replicated.  All heavy matmuls in bf16 with fp32 PSUM accumulation.

Device algorithm per core (all layouts transposed so D is on partitions):
  kT[dout, l]   = sum_k UT[k, dout] * encT[k, l]        (PE, bf16)
  tanh tiles    = tanh(kT + (q + W_b + U_b))            (ACT, bias per-partition)
  scores[1, l]  = sum_dout vT[dout] * tanh[dout, l]     (PE)
  exp row       = exp(scores)  (+ per-chunk sums via accum_out)  (ACT)
  expT[l, 1]    = tiny K=1 matmuls partition-izing the exp row   (PE)
  ctx[1, d]     = sum_l expT[l] * encN[l, d]            (PE)  then * 1/Z
  attn          = exp * 1/Z
v_b is dropped: softmax is invariant to a constant score offset.
exp without max-subtraction is safe: |scores| <~ 1.5 for this problem
(|scores| <= sum|v| = 32 in the absolute worst case; exp(32) is finite in fp32).
"""

import os
import sys

sys.path.insert(0, "/opt/trn_rl_repo")

import numpy as np
import ml_dtypes

from contextlib import ExitStack

import concourse.bass as bass
import concourse.mybir as mybir
import concourse.tile as tile
from concourse import bacc
from concourse.bass_utils import run_bass_kernel_spmd


def _install_ntff_hook_shim():
    """This image's ``antenv`` lacks ``axon_hooks``, so bass_utils' trace=True
    path crashes.  Register an equivalent module backed by the ctypes NTFF
    profiler in trn_agent_boot (libaxon_pjrt.so C ABI)."""
    import types
    import importlib.util

    if importlib.util.find_spec("antenv.axon_hooks") is not None:
        return
    try:
        if "/root/.axon_site" not in sys.path:
            sys.path.insert(0, "/root/.axon_site")
        from trn_agent_boot.trn_boot import _ntff_profile_via_ctypes

        hook = _ntff_profile_via_ctypes("/opt/axon/libaxon_pjrt.so")
    except Exception:
        hook = None
    mod = types.ModuleType("antenv.axon_hooks")
    mod._hook = hook
    mod.get_axon_ntff_profile_hook = lambda: mod._hook
    mod.set_axon_ntff_profile_hook = lambda h: setattr(mod, "_hook", h)
    sys.modules["antenv.axon_hooks"] = mod


_install_ntff_hook_shim()

BF16 = mybir.dt.bfloat16
F32 = mybir.dt.float32
NPBF16 = ml_dtypes.bfloat16

B, L, D = 32, 2048, 1024
NCORES = 8
NB = B // NCORES          # batches per core = 4
P = 128
KT = D // P               # 8 contraction tiles
MT = D // P               # 8 output-dim tiles
NCHUNK = 512              # matmul moving free dim / PSUM bank
CPB = L // NCHUNK         # 4 chunks per batch
LT = L // P               # 16 l-tiles per batch
ACT_TANH = mybir.ActivationFunctionType.Tanh
ACT_EXP = mybir.ActivationFunctionType.Exp


def build_nc() -> bass.Bass:
    # Bacc (not raw Bass): its compile() pass splits multi-sem waits into
    # event semaphores (TRN2 allows 1 embedded wait/instruction) and inserts
    # ACT table loads.
    nc = bacc.Bacc()

    encN = nc.dram_tensor("encN", [NB * L, D], BF16, kind="ExternalInput")
    encT = nc.dram_tensor("encT", [NB, D, L], BF16, kind="ExternalInput")
    WT = nc.dram_tensor("WT", [D, D], BF16, kind="ExternalInput")
    UT = nc.dram_tensor("UT", [D, D], BF16, kind="ExternalInput")
    decT = nc.dram_tensor("decT", [D, NB], BF16, kind="ExternalInput")
    biasWU = nc.dram_tensor("biasWU", [P, MT], F32, kind="ExternalInput")
    vT = nc.dram_tensor("vT", [P, MT], BF16, kind="ExternalInput")

    ctx_out = nc.dram_tensor("ctx", [NB, D], F32, kind="ExternalOutput")
    attn_out = nc.dram_tensor("attn", [NB, L], F32, kind="ExternalOutput")

    with tile.TileContext(nc) as tc, ExitStack() as ctx:
        consts = ctx.enter_context(tc.tile_pool(name="consts", bufs=1))
        io = ctx.enter_context(tc.tile_pool(name="io", bufs=2))
        big = ctx.enter_context(tc.tile_pool(name="big", bufs=3))
        psA = ctx.enter_context(tc.tile_pool(name="psA", bufs=4, space="PSUM"))
        psB = ctx.enter_context(tc.tile_pool(name="psB", bufs=3, space="PSUM"))
        psC = ctx.enter_context(tc.tile_pool(name="psC", bufs=1, space="PSUM"))

        # ---- constants -----------------------------------------------------
        UT_sb = consts.tile([P, KT, D], BF16, tag="UT")
        nc.sync.dma_start(UT_sb, UT.rearrange("(ko p) m -> p ko m", p=P))
        WT_sb = consts.tile([P, KT, D], BF16, tag="WT")
        nc.sync.dma_start(WT_sb, WT.rearrange("(ko p) m -> p ko m", p=P))
        decT_sb = consts.tile([P, KT, NB], BF16, tag="decT")
        nc.sync.dma_start(decT_sb, decT.rearrange("(ko p) b -> p ko b", p=P))
        bias_sb = consts.tile([P, MT], F32, tag="bias")
        nc.sync.dma_start(bias_sb, biasWU[:, :])
        vT_sb = consts.tile([P, MT], BF16, tag="vT")
        nc.sync.dma_start(vT_sb, vT[:, :])
        ones_sb = consts.tile([1, 1], BF16, tag="ones")
        nc.vector.memset(ones_sb, 1.0)

        # ---- q^T = W @ dec^T, then qb = q^T + (W_b + U_b) ------------------
        qb_sb = consts.tile([P, MT, NB], F32, tag="qb")
        for m in range(MT):
            ps_q = psB.tile([P, NCHUNK], F32, tag="row", name="ps_q")
            for k in range(KT):
                nc.tensor.matmul(
                    ps_q[:, :NB],
                    WT_sb[:, k, m * P : (m + 1) * P],
                    decT_sb[:, k, :],
                    start=(k == 0),
                    stop=(k == KT - 1),
                )
            # ScalarE Identity-with-bias: TensorScalarPtr on DVE only has one
            # sync-wait slot and this op needs two (PE + DMA).
            nc.scalar.activation(
                qb_sb[:, m, :],
                ps_q[:, :NB],
                mybir.ActivationFunctionType.Identity,
                bias=bias_sb[:, m : m + 1],
                scale=1.0,
            )

        # ---- per-batch state ----------------------------------------------
        exp_rows = {}
        expbf_rows = {}
        zparts = {}
        expT_ps = {}

        def emit_proj_scores(b: int, c: int):
            if c == 0:
                exp_rows[b] = io.tile([1, L], F32, tag="exp_row", name="exp_row")
                expbf_rows[b] = io.tile([1, L], BF16, tag="expbf", name="expbf")
                zparts[b] = io.tile([1, CPB], F32, tag="zpart", name="zpart")
                expT_ps[b] = psC.tile([P, LT], F32, tag="expT", name="expT_ps")

            et = big.tile([P, KT, NCHUNK], BF16, tag="encT", name="et")
            nc.sync.dma_start(
                et,
                encT[b].rearrange("(ko p) l -> p ko l", p=P)[
                    :, :, c * NCHUNK : (c + 1) * NCHUNK
                ],
            )
            th = big.tile([P, MT, NCHUNK], BF16, tag="tanh", name="th")
            for m in range(MT):
                pk = psA.tile([P, NCHUNK], F32, tag="kproj", name="pk")
                for k in range(KT):
                    nc.tensor.matmul(
                        pk,
                        UT_sb[:, k, m * P : (m + 1) * P],
                        et[:, k, :],
                        start=(k == 0),
                        stop=(k == KT - 1),
                    )
                nc.scalar.activation(
                    th[:, m, :], pk, ACT_TANH, bias=qb_sb[:, m, b : b + 1], scale=1.0
                )
            ss = psB.tile([1, NCHUNK], F32, tag="row", name="ss")
            for m in range(MT):
                nc.tensor.matmul(
                    ss,
                    vT_sb[:, m : m + 1],
                    th[:, m, :],
                    start=(m == 0),
                    stop=(m == MT - 1),
                )
            csl = slice(c * NCHUNK, (c + 1) * NCHUNK)
            nc.scalar.activation(
                exp_rows[b][:, csl],
                ss,
                ACT_EXP,
                bias=0.0,
                scale=1.0,
                accum_out=zparts[b][:, c : c + 1],
            )
            nc.vector.tensor_copy(expbf_rows[b][:, csl], exp_rows[b][:, csl])

        def emit_expT(b: int, c: int):
            # partition-ize exp chunk: 4 tiny K=1 matmuls -> expT psum columns
            for t4 in range(NCHUNK // P):
                t = c * (NCHUNK // P) + t4
                nc.tensor.matmul(
                    expT_ps[b][:, t : t + 1],
                    expbf_rows[b][0:1, t * P : (t + 1) * P],
                    ones_sb[0:1, 0:1],
                    start=True,
                    stop=True,
                )

        def emit_ctx(b: int):
            z = io.tile([1, 1], F32, tag="z", name="z")
            nc.vector.reduce_sum(z, zparts[b], axis=mybir.AxisListType.X)
            rz = io.tile([1, 1], F32, tag="rz", name="rz")
            nc.vector.reciprocal(rz, z)

            attn_sb = io.tile([1, L], F32, tag="attn_sb", name="attn_sb")
            nc.vector.tensor_mul(attn_sb, exp_rows[b], rz.to_broadcast((1, L)))
            nc.sync.dma_start(attn_out[b : b + 1, :], attn_sb)

            expT_sb = io.tile([P, LT], BF16, tag="expT_sb", name="expT_sb")
            nc.vector.tensor_copy(expT_sb, expT_ps[b])

            pc0 = psB.tile([1, NCHUNK], F32, tag="row", name="pc0")
            pc1 = psB.tile([1, NCHUNK], F32, tag="row", name="pc1")
            for g in range(LT // 4):
                en = big.tile([P, 4, D], BF16, tag="encN", name="en", bufs=5)
                nc.sync.dma_start(
                    en,
                    encN[b * L + g * 4 * P : b * L + (g + 1) * 4 * P].rearrange(
                        "(g2 p) d -> p g2 d", p=P
                    ),
                )
                for i in range(4):
                    t = g * 4 + i
                    nc.tensor.matmul(
                        pc0,
                        expT_sb[:, t : t + 1],
                        en[:, i, 0:NCHUNK],
                        start=(t == 0),
                        stop=(t == LT - 1),
                    )
                    nc.tensor.matmul(
                        pc1,
                        expT_sb[:, t : t + 1],
                        en[:, i, NCHUNK:D],
                        start=(t == 0),
                        stop=(t == LT - 1),
                    )
            ctx_sb = io.tile([1, D], F32, tag="ctx_sb", name="ctx_sb")
            nc.vector.tensor_mul(ctx_sb[:, 0:NCHUNK], pc0, rz.to_broadcast((1, NCHUNK)))
            nc.vector.tensor_mul(ctx_sb[:, NCHUNK:D], pc1, rz.to_broadcast((1, NCHUNK)))
            nc.sync.dma_start(ctx_out[b : b + 1, :], ctx_sb)

        # ---- main pipeline: delay expT by one chunk, ctx(b) after ----------
        # proj(b+1, c0) so the PE never FIFO-blocks on ACT/DVE latency.
        pending_expT = []
        pending_ctx = []
        for b in range(NB):
            for c in range(CPB):
                emit_proj_scores(b, c)
                if pending_expT:
                    pending_expT.pop(0)()
                if c == 1 and pending_ctx:
                    pending_ctx.pop(0)()
                pending_expT.append(lambda b=b, c=c: emit_expT(b, c))
            pending_ctx.append(lambda b=b: emit_ctx(b))
        while pending_expT:
            pending_expT.pop(0)()
        while pending_ctx:
            pending_ctx.pop(0)()

    nc.finalize()  # Bacc: runs compile() (wait-splitting, reg alloc, ACT tables)
    return nc


def prep_in_maps(decoder_hidden, encoder_outputs, W_w, W_b, U_w, U_b, v_w, v_b):
    """Host-side shard + layout prep (numpy). All FLOPs stay on device."""
    dec = np.asarray(decoder_hidden, dtype=np.float32)
    enc = np.asarray(encoder_outputs, dtype=np.float32)
    W_w = np.asarray(W_w, dtype=np.float32)
    W_b = np.asarray(W_b, dtype=np.float32)
    U_w = np.asarray(U_w, dtype=np.float32)
    U_b = np.asarray(U_b, dtype=np.float32)
    v_w = np.asarray(v_w, dtype=np.float32)

    WT = np.ascontiguousarray(W_w.T).astype(NPBF16)          # [D, D] = W^T
    UT = np.ascontiguousarray(U_w.T).astype(NPBF16)          # [D, D] = U^T
    biasWU = np.ascontiguousarray((W_b + U_b).reshape(MT, P).T).astype(np.float32)
    vTa = np.ascontiguousarray(v_w[0].reshape(MT, P).T).astype(NPBF16)

    in_maps = []
    for i in range(NCORES):
        sl = slice(i * NB, (i + 1) * NB)
        enc_i = enc[sl]                                       # [NB, L, D]
        in_maps.append(
            {
                "encN": np.ascontiguousarray(enc_i.reshape(NB * L, D)).astype(NPBF16),
                "encT": np.ascontiguousarray(enc_i.transpose(0, 2, 1)).astype(NPBF16),
                "WT": WT,
                "UT": UT,
                "decT": np.ascontiguousarray(dec[sl, 0, :].T).astype(NPBF16),
                "biasWU": biasWU,
                "vT": vTa,
            }
        )
    return in_maps


_NC_CACHE = None


def _get_nc():
    global _NC_CACHE
    if _NC_CACHE is None:
        _NC_CACHE = build_nc()
    return _NC_CACHE


def run(inputs: dict, trace: bool = False):
    """Returns ((context, attn), BassKernelResults)."""
    nc = _get_nc()
    in_maps = prep_in_maps(**inputs)
    res = run_bass_kernel_spmd(
        nc, in_maps, core_ids=list(range(NCORES)), trace=trace
    )
    context = np.zeros((B, 1, D), dtype=np.float32)
    attn = np.zeros((B, 1, L), dtype=np.float32)
    for i, r in enumerate(res.results):
        context[i * NB : (i + 1) * NB, 0, :] = r["ctx"]
        attn[i * NB : (i + 1) * NB, 0, :] = r["attn"]
    return (context, attn), res


def kernel(**inputs):
    out, _ = run(inputs, trace=False)
    return out


if __name__ == "__main__":
    rng = np.random.default_rng(0)
    fake = {
        "decoder_hidden": rng.standard_normal((B, 1, D), dtype=np.float32),
        "encoder_outputs": rng.standard_normal((B, L, D), dtype=np.float32),
        "W_w": rng.uniform(-0.03, 0.03, (D, D)).astype(np.float32),
        "W_b": rng.uniform(-0.03, 0.03, (D,)).astype(np.float32),
        "U_w": rng.uniform(-0.03, 0.03, (D, D)).astype(np.float32),
        "U_b": rng.uniform(-0.03, 0.03, (D,)).astype(np.float32),
        "v_w": rng.uniform(-0.03, 0.03, (1, D)).astype(np.float32),
        "v_b": rng.uniform(-0.03, 0.03, (1,)).astype(np.float32),
    }
    (ctx_o, attn_o), _ = run(fake)
    print("ok", ctx_o.shape, attn_o.shape)
